# revision 4
# baseline (speedup 1.0000x reference)
"""GQA attention (32 q heads / 8 kv heads, D=64, HID=2048, B=2, T=2048)
distributed over 8 TRN2 NeuronCores.

Sharding: 2-way data parallel (batch) x 4-way tensor parallel (head groups).
Core c handles batch c//4 and head group g=c%4 (q heads [8g,8g+8), kv heads
[2g,2g+2)).  Each core projects Q^T/K^T (transposed layout: head-dims on
partitions, T on free axis) and V (T on partitions), applies RoPE, computes
scores^T = K @ Q^T per head with keys on partitions, exp via ScalarE
(no max-subtraction needed at these magnitudes; masked entries multiply to
exactly 0 by a host-precomputed exp(mask) factor, and all-zero /all-one mask
tiles are specialized at graph-build time from the actual mask input), then
out^T = Vext^T @ P^T where Vext carries a ones column producing the softmax
denominators for free.  The per-group AllGather of the transposed attention
output feeds a local o_proj slice (512 output columns per core).

Everything matmul-facing is bf16 with fp32 PSUM accumulation.
"""

import os
import numpy as np
import ml_dtypes

BF16 = ml_dtypes.bfloat16

HQ, HKV, D, HID, THETA = 32, 8, 64, 2048, 10000.0
NCORES, NGROUPS = 8, 4
QDIM = HQ * D // NGROUPS        # 512 q dims per core
KVDIM = HKV * D // NGROUPS      # 128 kv dims per core
NQT = 512                       # query tile (free dim per PSUM bank)
NKC = 128                       # key chunk (partition dim)

_cache = {}
LAST_RESULT = None              # BassKernelResults of the most recent run


def plan_mask(mask, T):
    """Classify (key-chunk i, q-tile j) tiles of exp(mask).T.

    Returns (plans, emt_tiles): plans[j] = list of (i, kind, emt_idx) where
    kind 0 = no mask needed (exp(mask)==1 on tile), kind 1 = multiply by
    emt_tiles[emt_idx].  All-zero tiles are skipped entirely (they contribute
    nothing to P@V nor to the softmax denominator).
    """
    m = np.asarray(mask, dtype=np.float32).reshape(T, T)
    with np.errstate(under="ignore"):
        em = np.exp(m).T.astype(np.float32)   # em[k, q] = exp(mask[q, k])
    nj, nk = T // NQT, T // NKC
    plans, emt_tiles = [], []
    for j in range(nj):
        pj = []
        for i in range(nk):
            t = em[i * NKC:(i + 1) * NKC, j * NQT:(j + 1) * NQT]
            if not t.any():
                continue
            if (t == 1.0).all():
                pj.append((i, 0, -1))
            else:
                pj.append((i, 1, len(emt_tiles)))
                emt_tiles.append(t.astype(BF16))
        plans.append(pj)
    return plans, emt_tiles


def build_graph(T, plans, n_emt):
    """Build the SPMD Bacc graph (same on all 8 cores; shards arrive as data)."""
    import concourse.bass as bass  # noqa: F401
    import concourse.mybir as mybir
    import concourse.tile as tile
    from concourse import bacc

    f32, bf16 = mybir.dt.float32, mybir.dt.bfloat16
    AF, ALU = mybir.ActivationFunctionType, mybir.AluOpType

    nj = T // NQT          # q tiles
    ntb = T // NQT         # T blocks in projection
    nhc = HID // 128       # contraction chunks over hidden dim
    noc = (HQ * D) // 128  # contraction chunks over gathered head dim (16)

    nc = bacc.Bacc("TRN2", target_bir_lowering=False, debug=False,
                   num_devices=NCORES)

    xt = nc.dram_tensor("xt", [HID, T], bf16, kind="ExternalInput").ap()
    wqt = nc.dram_tensor("wqt", [HID, QDIM], bf16, kind="ExternalInput").ap()
    wkt = nc.dram_tensor("wkt", [HID, KVDIM], bf16, kind="ExternalInput").ap()
    wvt = nc.dram_tensor("wvt", [HID, KVDIM], bf16, kind="ExternalInput").ap()
    qb = nc.dram_tensor("qb", [4, 128, 1], f32, kind="ExternalInput").ap()
    kb = nc.dram_tensor("kb", [1, 128, 1], f32, kind="ExternalInput").ap()
    vb = nc.dram_tensor("vb", [1, KVDIM], bf16, kind="ExternalInput").ap()
    cosq = nc.dram_tensor("cosq", [128, T], f32, kind="ExternalInput").ap()
    ssin = nc.dram_tensor("ssin", [128, T], f32, kind="ExternalInput").ap()
    emt = nc.dram_tensor("emt", [max(n_emt, 1), NKC, NQT], bf16,
                         kind="ExternalInput").ap()
    owt = nc.dram_tensor("owt", [HQ * D, QDIM], bf16, kind="ExternalInput").ap()
    out = nc.dram_tensor("out", [T, QDIM], f32, kind="ExternalOutput").ap()

    rg = [[0, 1, 2, 3], [4, 5, 6, 7]]

    with tile.TileContext(nc) as tc:
        with tc.tile_pool(name="dramp", bufs=1, space="DRAM") as dramp:
            ag_in = [dramp.tile([QDIM, NQT], bf16, name=f"agin{j}")
                     for j in range(nj)]
            # Shared addr_space is rejected for 4-core groups; Local works
            # (HBM-HBM path, slight perf warning for >1MB).
            ag_out = [dramp.tile([NGROUPS * QDIM, NQT], bf16,
                                 name=f"agout{j}")
                      for j in range(nj)]

        with tc.tile_pool(name="persist", bufs=1) as pp:
            # Q^T per head-pair chunk: [128 (2 heads x 64), T]
            qt = [pp.tile([128, T], bf16, name=f"qt{m}") for m in range(4)]
            # K^T duplicated per kv head: [128 = kv dup'd twice, T]
            ktd = [pp.tile([128, T], bf16, name=f"ktd{k}") for k in range(2)]
            # V per key chunk: [128 keys, 130] (cols 0:64 kv0|64 ones|65:129 kv1|129 ones)
            vsb = [pp.tile([128, 130], bf16, name=f"v{i}")
                   for i in range(T // NKC)]
            ow_sb = pp.tile([128, noc, QDIM], bf16, name="ow_sb")
            vb_sb = pp.tile([1, KVDIM], bf16, name="vb_sb")
            ones_sb = pp.tile([1, 128], bf16, name="ones_sb")
            qb_sb = pp.tile([128, 4], f32, name="qb_sb")
            kb_sb = pp.tile([128, 1], f32, name="kb_sb")

            nc.sync.dma_start(out=ow_sb[:],
                              in_=owt.rearrange("(c p) q -> p c q", p=128))
            nc.sync.dma_start(out=vb_sb[:], in_=vb)
            nc.vector.memset(ones_sb[:], 1.0)
            nc.sync.dma_start(out=qb_sb[:], in_=qb.rearrange("c p 1 -> p c"))
            nc.sync.dma_start(out=kb_sb[:], in_=kb.rearrange("c p 1 -> p c"))

            # ---------------- projections + RoPE ----------------
            with tc.tile_pool(name="projw", bufs=1) as pw, \
                 tc.tile_pool(name="projx", bufs=2) as px, \
                 tc.tile_pool(name="projps", bufs=2, space="PSUM") as pps, \
                 tc.tile_pool(name="projtmp", bufs=3) as ptmp:
                wq_sb = pw.tile([128, nhc, QDIM], bf16, name="wq_sb")
                wk_sb = pw.tile([128, nhc, KVDIM], bf16, name="wk_sb")
                wv_sb = pw.tile([128, nhc, KVDIM], bf16, name="wv_sb")
                cos_sb = pw.tile([128, T], f32, name="cos_sb")
                ssin_sb = pw.tile([128, T], f32, name="ssin_sb")
                nc.sync.dma_start(out=wq_sb[:],
                                  in_=wqt.rearrange("(c p) q -> p c q", p=128))
                nc.sync.dma_start(out=wk_sb[:],
                                  in_=wkt.rearrange("(c p) q -> p c q", p=128))
                nc.sync.dma_start(out=wv_sb[:],
                                  in_=wvt.rearrange("(c p) q -> p c q", p=128))
                nc.sync.dma_start(out=cos_sb[:], in_=cosq)
                nc.sync.dma_start(out=ssin_sb[:], in_=ssin)

                xt_r = xt.rearrange("(c p) t -> p c t", p=128)

                def rope_evict(ps, bias_col, dst, ts):
                    """dst[:, ts] = RoPE(ps + bias) cast to bf16."""
                    t2 = ptmp.tile([128, NQT], f32, name="t2", tag="t2")
                    nc.vector.scalar_tensor_tensor(
                        t2[:], ps[:], bias_col, ssin_sb[:, ts],
                        op0=ALU.add, op1=ALU.mult)
                    t2s = ptmp.tile([128, NQT], f32, name="t2s", tag="t2s")
                    for blk in range(4):
                        sb = blk ^ 1
                        nc.sync.dma_start(
                            out=t2s[blk * 32:(blk + 1) * 32, :],
                            in_=t2[sb * 32:(sb + 1) * 32, :])
                    t1 = ptmp.tile([128, NQT], f32, name="t1", tag="t1")
                    nc.vector.scalar_tensor_tensor(
                        t1[:], ps[:], bias_col, cos_sb[:, ts],
                        op0=ALU.add, op1=ALU.mult)
                    nc.vector.tensor_add(dst, t1[:], t2s[:])

                for tb in range(ntb):
                    ts = slice(tb * NQT, (tb + 1) * NQT)
                    x_sb = px.tile([128, nhc, NQT], bf16, name="x_sb")
                    nc.sync.dma_start(out=x_sb[:], in_=xt_r[:, :, ts])
                    # Q^T chunks (4 x [128, NQT])
                    for m in range(4):
                        ps = pps.tile([128, NQT], f32, name="ps", tag="ps")
                        for c in range(nhc):
                            nc.tensor.matmul(
                                ps[:], wq_sb[:, c, m * 128:(m + 1) * 128],
                                x_sb[:, c, :],
                                start=(c == 0), stop=(c == nhc - 1))
                        rope_evict(ps, qb_sb[:, m:m + 1], qt[m][:, ts], ts)
                    # K^T (one [128, NQT] chunk = 2 kv heads)
                    psk = pps.tile([128, NQT], f32, name="psk", tag="ps")
                    for c in range(nhc):
                        nc.tensor.matmul(psk[:], wk_sb[:, c, :], x_sb[:, c, :],
                                         start=(c == 0), stop=(c == nhc - 1))
                    kf = ptmp.tile([128, NQT], bf16, name="kf", tag="kf")
                    rope_evict(psk, kb_sb[:, 0:1], kf[:], ts)
                    for half in (0, 1):           # kv head within chunk
                        for dsth in (0, 1):       # duplicate to both halves
                            nc.sync.dma_start(
                                out=ktd[half][dsth * 64:(dsth + 1) * 64, ts],
                                in_=kf[half * 64:(half + 1) * 64, :])
                    # V (T rows on partitions) + bias via rank-1 matmul
                    for st in range(NQT // 128):
                        psv = pps.tile([128, KVDIM], f32, name="psv", tag="psv")
                        for c in range(nhc):
                            nc.tensor.matmul(
                                psv[:], x_sb[:, c, st * 128:(st + 1) * 128],
                                wv_sb[:, c, :],
                                start=(c == 0), stop=False)
                        nc.tensor.matmul(psv[:], ones_sb[:], vb_sb[:],
                                         start=False, stop=True)
                        vi = tb * (NQT // 128) + st
                        nc.vector.memset(vsb[vi][:, :], 1.0)
                        nc.vector.tensor_copy(vsb[vi][:, 0:64], psv[:, 0:64])
                        nc.vector.tensor_copy(vsb[vi][:, 65:129], psv[:, 64:128])

            # ---------------- attention + AllGather + o_proj ----------------
            max_emt_j = max((sum(1 for e in pj if e[1] == 1) for pj in plans),
                            default=1)
            with tc.tile_pool(name="emtp", bufs=max(2, min(2 * max_emt_j, 12))) as ep, \
                 tc.tile_pool(name="ptp", bufs=3) as ptp, \
                 tc.tile_pool(name="psS", bufs=2, space="PSUM") as psS, \
                 tc.tile_pool(name="psO", bufs=1, space="PSUM") as psO, \
                 tc.tile_pool(name="evp", bufs=4) as evp, \
                 tc.tile_pool(name="psF", bufs=2, space="PSUM") as psF, \
                 tc.tile_pool(name="agp", bufs=2) as agp, \
                 tc.tile_pool(name="outp", bufs=4) as outp:
                for j in range(nj):
                    qs = slice(j * NQT, (j + 1) * NQT)
                    emt_sb = {}
                    for (i, kind, gi) in plans[j]:
                        if kind == 1:
                            e = ep.tile([NKC, NQT], bf16, name="emt_sb",
                                        tag="emt")
                            nc.sync.dma_start(out=e[:], in_=emt[gi])
                            emt_sb[i] = e
                    for hp in range(4):           # head pair (2hp, 2hp+1)
                        kv = hp // 2              # local kv head
                        po0 = psO.tile([65, NQT], f32, name="po0", tag="po0")
                        po1 = psO.tile([65, NQT], f32, name="po1", tag="po1")
                        n_ch = len(plans[j])
                        for ci, (i, kind, gi) in enumerate(plans[j]):
                            pss = psS.tile([128, 1024], f32, name="pss",
                                           tag="pss")
                            # head-lo on array rows 0:64, head-hi on 64:128 —
                            # concurrent row-groups, separate PSUM banks
                            nc.tensor.matmul(
                                pss[:, 0:512],
                                ktd[kv][0:64, i * NKC:(i + 1) * NKC],
                                qt[hp][0:64, qs], start=True, stop=True)
                            nc.tensor.matmul(
                                pss[:, 512:1024],
                                ktd[kv][64:128, i * NKC:(i + 1) * NKC],
                                qt[hp][64:128, qs], start=True, stop=True)
                            pt = ptp.tile([128, 1024], bf16, name="pt",
                                          tag="pt")
                            nc.scalar.activation(pt[:], pss[:], AF.Exp,
                                                 scale=0.125)
                            if kind == 1:
                                nc.vector.tensor_mul(pt[:, 0:512],
                                                     pt[:, 0:512], emt_sb[i][:])
                                nc.vector.tensor_mul(pt[:, 512:1024],
                                                     pt[:, 512:1024],
                                                     emt_sb[i][:])
                            vsl = vsb[i][:, 0:65] if kv == 0 else vsb[i][:, 65:130]
                            nc.tensor.matmul(po0[:], vsl, pt[:, 0:512],
                                             start=(ci == 0),
                                             stop=(ci == n_ch - 1))
                            nc.tensor.matmul(po1[:], vsl, pt[:, 512:1024],
                                             start=(ci == 0),
                                             stop=(ci == n_ch - 1))
                        for s, po in enumerate((po0, po1)):
                            rc = evp.tile([1, NQT], f32, name="rc", tag="rc")
                            nc.vector.reciprocal(rc[:], po[64:65, :])
                            rb = evp.tile([64, NQT], f32, name="rb", tag="rb")
                            nc.gpsimd.partition_broadcast(rb[:], rc[:])
                            at = evp.tile([64, NQT], bf16, name="at", tag="at")
                            nc.vector.tensor_mul(at[:], po[0:64, :], rb[:])
                            h = 2 * hp + s
                            nc.sync.dma_start(
                                out=ag_in[j][h * 64:(h + 1) * 64, :],
                                in_=at[:])
                    nc.gpsimd.collective_compute(
                        "AllGather", ALU.bypass, replica_groups=rg,
                        ins=[ag_in[j].opt()], outs=[ag_out[j].opt()])
                    ag_sb = agp.tile([128, noc, NQT], bf16, name="ag_sb")
                    nc.sync.dma_start(
                        out=ag_sb[:],
                        in_=ag_out[j].rearrange("(c p) t -> p c t", p=128))
                    for tt in range(NQT // 128):
                        pf = psF.tile([128, QDIM], f32, name="pf", tag="pf")
                        for c in range(noc):
                            nc.tensor.matmul(
                                pf[:], ag_sb[:, c, tt * 128:(tt + 1) * 128],
                                ow_sb[:, c, :],
                                start=(c == 0), stop=(c == noc - 1))
                        ot = outp.tile([128, QDIM], f32, name="ot", tag="ot")
                        nc.vector.tensor_copy(ot[:], pf[:])
                        nc.sync.dma_start(
                            out=out[j * NQT + tt * 128:
                                    j * NQT + (tt + 1) * 128, :],
                            in_=ot[:])

    nc.compile()
    return nc


def prep_inputs(hidden, positions, mask, q_w, q_b, k_w, k_b, v_w, v_b, o_w,
                emt_tiles):
    """Host-side shard + transform → in_maps for the 8 cores."""
    B, T, _ = hidden.shape
    pos = np.asarray(positions)[0].astype(np.float32)
    inv_freq = (1.0 / (THETA ** (np.arange(0, D, 2, dtype=np.float32) / D)))
    freqs = pos[:, None] * inv_freq[None, :]          # (T, 32)
    cos_t, sin_t = np.cos(freqs).T, np.sin(freqs).T   # (32, T)
    cos_tab = np.ascontiguousarray(np.tile(cos_t, (4, 1)), dtype=np.float32)
    ssin_tab = np.ascontiguousarray(
        np.concatenate([sin_t, -sin_t, sin_t, -sin_t], axis=0),
        dtype=np.float32)

    if emt_tiles:
        emt_arr = np.stack(emt_tiles).astype(BF16)
    else:
        emt_arr = np.zeros((1, NKC, NQT), BF16)

    xts = [np.ascontiguousarray(hidden[b].T).astype(BF16) for b in range(B)]
    in_maps = []
    for c in range(NCORES):
        b, g = c // NGROUPS, c % NGROUPS
        qsl = slice(QDIM * g, QDIM * (g + 1))
        ksl = slice(KVDIM * g, KVDIM * (g + 1))
        in_maps.append({
            "xt": xts[b],
            "wqt": np.ascontiguousarray(q_w[qsl, :].T).astype(BF16),
            "wkt": np.ascontiguousarray(k_w[ksl, :].T).astype(BF16),
            "wvt": np.ascontiguousarray(v_w[ksl, :].T).astype(BF16),
            "qb": np.asarray(q_b[qsl], np.float32).reshape(4, 128, 1),
            "kb": np.asarray(k_b[ksl], np.float32).reshape(1, 128, 1),
            "vb": np.asarray(v_b[ksl]).astype(BF16).reshape(1, KVDIM),
            "cosq": cos_tab,
            "ssin": ssin_tab,
            "emt": emt_arr,
            "owt": np.ascontiguousarray(o_w.T[:, qsl]).astype(BF16),
        })
    return in_maps


def _ensure_ntff_hook():
    """Provide antenv.axon_hooks in containers whose antenv stub lacks it,
    wiring the ctypes NTFF profiler from the injected axon boot package."""
    import sys
    import types
    try:
        from antenv.axon_hooks import get_axon_ntff_profile_hook  # noqa: F401
        return True
    except ImportError:
        pass
    try:
        import antenv
        from trn_agent_boot.trn_boot import _ntff_profile_via_ctypes
        hook = _ntff_profile_via_ctypes("/opt/axon/libaxon_pjrt.so")
        if hook is None:
            return False
        mod = types.ModuleType("antenv.axon_hooks")
        state = {"h": hook}
        mod.get_axon_ntff_profile_hook = lambda: state["h"]
        mod.set_axon_ntff_profile_hook = lambda h: state.__setitem__("h", h)
        sys.modules["antenv.axon_hooks"] = mod
        antenv.axon_hooks = mod
        return True
    except Exception:
        return False


def kernel(hidden, positions, mask, q_w, q_b, k_w, k_b, v_w, v_b, o_w):
    global LAST_RESULT
    from concourse import bass_utils

    hidden = np.asarray(hidden)
    B, T, _ = hidden.shape
    mask_key = (T, hash(np.asarray(mask).tobytes()))
    if mask_key not in _cache:
        plans, emt_tiles = plan_mask(mask, T)
        nc = build_graph(T, plans, len(emt_tiles))
        _cache[mask_key] = (nc, emt_tiles)
    nc, emt_tiles = _cache[mask_key]

    in_maps = prep_inputs(hidden, positions, mask, q_w, q_b, k_w, k_b,
                          v_w, v_b, o_w, emt_tiles)
    trace = os.environ.get("BASS_KERNEL_TRACE", "0") == "1"
    if trace:
        trace = _ensure_ntff_hook()
    res = bass_utils.run_bass_kernel_spmd(nc, in_maps,
                                          core_ids=list(range(NCORES)),
                                          trace=trace)
    LAST_RESULT = res
    out = np.zeros((B, T, HID), np.float32)
    for c in range(NCORES):
        b, g = c // NGROUPS, c % NGROUPS
        out[b, :, QDIM * g:QDIM * (g + 1)] = res.results[c]["out"]
    return out


# revision 9
# speedup vs baseline: 1.1320x; 1.1320x over previous
"""GQA attention (32 q heads / 8 kv heads, D=64, HID=2048, B=2, T=2048)
distributed over 8 TRN2 NeuronCores.

Sharding: 2-way data parallel (batch) x 4-way tensor parallel (head groups).
Core c handles batch c//4 and head group g=c%4 (q heads [8g,8g+8), kv heads
[2g,2g+2)).  Each core projects Q^T/K^T (transposed layout: head-dims on
partitions, T on free axis) and V (T on partitions), applies RoPE, computes
scores^T = K @ Q^T per head with keys on partitions, exp via ScalarE
(no max-subtraction needed at these magnitudes; masked entries multiply to
exactly 0 by a host-precomputed exp(mask) factor, and all-zero /all-one mask
tiles are specialized at graph-build time from the actual mask input), then
out^T = Vext^T @ P^T where Vext carries a ones column producing the softmax
denominators for free.  The per-group AllGather of the transposed attention
output feeds a local o_proj slice (512 output columns per core).

Everything matmul-facing is bf16 with fp32 PSUM accumulation.
"""

import os
import numpy as np
import ml_dtypes

BF16 = ml_dtypes.bfloat16

HQ, HKV, D, HID, THETA = 32, 8, 64, 2048, 10000.0
NCORES, NGROUPS = 8, 4
QDIM = HQ * D // NGROUPS        # 512 q dims per core
KVDIM = HKV * D // NGROUPS      # 128 kv dims per core
NQT = 512                       # query tile (free dim per PSUM bank)
NKC = 128                       # key chunk (partition dim)

_cache = {}
LAST_RESULT = None              # BassKernelResults of the most recent run


def plan_mask(mask, T):
    """Classify (key-chunk i, q-tile j) tiles of exp(mask).T.

    Returns (plans, emt_tiles): plans[j] = list of (i, kind, emt_idx) where
    kind 0 = no mask needed (exp(mask)==1 on tile), kind 1 = multiply by
    emt_tiles[emt_idx].  All-zero tiles are skipped entirely (they contribute
    nothing to P@V nor to the softmax denominator).
    """
    m = np.asarray(mask, dtype=np.float32).reshape(T, T)
    with np.errstate(under="ignore"):
        em = np.exp(m).T.astype(np.float32)   # em[k, q] = exp(mask[q, k])
    nj, nk = T // NQT, T // NKC
    plans, emt_tiles = [], []
    for j in range(nj):
        pj = []
        for i in range(nk):
            t = em[i * NKC:(i + 1) * NKC, j * NQT:(j + 1) * NQT]
            if not t.any():
                continue
            if (t == 1.0).all():
                pj.append((i, 0, -1))
            else:
                pj.append((i, 1, len(emt_tiles)))
                emt_tiles.append(t.astype(BF16))
        plans.append(pj)
    return plans, emt_tiles


def build_graph(T, plans, n_emt):
    """Build the SPMD Bacc graph (same on all 8 cores; shards arrive as data)."""
    import concourse.bass as bass  # noqa: F401
    import concourse.mybir as mybir
    import concourse.tile as tile
    from concourse import bacc

    f32, bf16 = mybir.dt.float32, mybir.dt.bfloat16
    AF, ALU = mybir.ActivationFunctionType, mybir.AluOpType

    nj = T // NQT          # q tiles
    ntb = T // NQT         # T blocks in projection
    nhc = HID // 128       # contraction chunks over hidden dim
    noc = (HQ * D) // 128  # contraction chunks over gathered head dim (16)

    nc = bacc.Bacc("TRN2", target_bir_lowering=False, debug=False,
                   num_devices=NCORES)

    xt = nc.dram_tensor("xt", [HID, T], bf16, kind="ExternalInput").ap()
    wqt = nc.dram_tensor("wqt", [HID, QDIM], bf16, kind="ExternalInput").ap()
    wkt = nc.dram_tensor("wkt", [HID, KVDIM], bf16, kind="ExternalInput").ap()
    wvt = nc.dram_tensor("wvt", [HID, KVDIM], bf16, kind="ExternalInput").ap()
    qb = nc.dram_tensor("qb", [4, 128, 1], f32, kind="ExternalInput").ap()
    kb = nc.dram_tensor("kb", [1, 128, 1], f32, kind="ExternalInput").ap()
    vb = nc.dram_tensor("vb", [1, KVDIM], bf16, kind="ExternalInput").ap()
    cosq = nc.dram_tensor("cosq", [128, T], f32, kind="ExternalInput").ap()
    ssin = nc.dram_tensor("ssin", [128, T], f32, kind="ExternalInput").ap()
    emt = nc.dram_tensor("emt", [max(n_emt, 1), NKC, NQT], bf16,
                         kind="ExternalInput").ap()
    owt = nc.dram_tensor("owt", [HQ * D, QDIM], bf16, kind="ExternalInput").ap()
    out = nc.dram_tensor("out", [T, QDIM], f32, kind="ExternalOutput").ap()

    rg = [[0, 1, 2, 3], [4, 5, 6, 7]]

    with tile.TileContext(nc) as tc:
        with tc.tile_pool(name="dramp", bufs=1, space="DRAM") as dramp:
            ag_in = [dramp.tile([QDIM, NQT], bf16, name=f"agin{j}")
                     for j in range(nj)]
            # Shared addr_space is rejected for 4-core groups; Local works
            # (HBM-HBM path, slight perf warning for >1MB).
            ag_out = [dramp.tile([NGROUPS * QDIM, NQT], bf16,
                                 name=f"agout{j}")
                      for j in range(nj)]

        with tc.tile_pool(name="persist", bufs=1) as pp:
            # Q^T per head-pair chunk: [128 (2 heads x 64), T]
            qt = [pp.tile([128, T], bf16, name=f"qt{m}") for m in range(4)]
            # K^T duplicated per kv head: [128 = kv dup'd twice, T]
            ktd = [pp.tile([128, T], bf16, name=f"ktd{k}") for k in range(2)]
            # V per key chunk: [128 keys, 130] (cols 0:64 kv0|64 ones|65:129 kv1|129 ones)
            vsb = [pp.tile([128, 130], bf16, name=f"v{i}")
                   for i in range(T // NKC)]
            ow_sb = pp.tile([128, noc, QDIM], bf16, name="ow_sb")
            vb_sb = pp.tile([1, KVDIM], bf16, name="vb_sb")
            ones_sb = pp.tile([1, 128], bf16, name="ones_sb")
            qb_sb = pp.tile([128, 4], f32, name="qb_sb")
            kb_sb = pp.tile([128, 1], f32, name="kb_sb")

            nc.sync.dma_start(out=ow_sb[:],
                              in_=owt.rearrange("(c p) q -> p c q", p=128))
            nc.sync.dma_start(out=vb_sb[:], in_=vb)
            nc.vector.memset(ones_sb[:], 1.0)
            nc.sync.dma_start(out=qb_sb[:], in_=qb.rearrange("c p 1 -> p c"))
            nc.sync.dma_start(out=kb_sb[:], in_=kb.rearrange("c p 1 -> p c"))

            # ---------------- projections + RoPE ----------------
            with tc.tile_pool(name="projw", bufs=1) as pw, \
                 tc.tile_pool(name="projx", bufs=2) as px, \
                 tc.tile_pool(name="projps", bufs=2, space="PSUM") as pps, \
                 tc.tile_pool(name="projtmp", bufs=3) as ptmp:
                wq_sb = pw.tile([128, nhc, QDIM], bf16, name="wq_sb")
                wk_sb = pw.tile([128, nhc, KVDIM], bf16, name="wk_sb")
                wv_sb = pw.tile([128, nhc, KVDIM], bf16, name="wv_sb")
                cos_sb = pw.tile([128, T], f32, name="cos_sb")
                ssin_sb = pw.tile([128, T], f32, name="ssin_sb")
                nc.sync.dma_start(out=wq_sb[:],
                                  in_=wqt.rearrange("(c p) q -> p c q", p=128))
                nc.sync.dma_start(out=wk_sb[:],
                                  in_=wkt.rearrange("(c p) q -> p c q", p=128))
                nc.sync.dma_start(out=wv_sb[:],
                                  in_=wvt.rearrange("(c p) q -> p c q", p=128))
                nc.sync.dma_start(out=cos_sb[:], in_=cosq)
                nc.sync.dma_start(out=ssin_sb[:], in_=ssin)

                xt_r = xt.rearrange("(c p) t -> p c t", p=128)

                def rope_evict(ps, bias_col, dst, ts):
                    """dst[:, ts] = RoPE(ps + bias) cast to bf16."""
                    t2 = ptmp.tile([128, NQT], f32, name="t2", tag="t2")
                    nc.vector.scalar_tensor_tensor(
                        t2[:], ps[:], bias_col, ssin_sb[:, ts],
                        op0=ALU.add, op1=ALU.mult)
                    t2s = ptmp.tile([128, NQT], f32, name="t2s", tag="t2s")
                    for blk in range(4):
                        sb = blk ^ 1
                        nc.sync.dma_start(
                            out=t2s[blk * 32:(blk + 1) * 32, :],
                            in_=t2[sb * 32:(sb + 1) * 32, :])
                    t1 = ptmp.tile([128, NQT], f32, name="t1", tag="t1")
                    nc.vector.scalar_tensor_tensor(
                        t1[:], ps[:], bias_col, cos_sb[:, ts],
                        op0=ALU.add, op1=ALU.mult)
                    nc.vector.tensor_add(dst, t1[:], t2s[:])

                for tb in range(ntb):
                    ts = slice(tb * NQT, (tb + 1) * NQT)
                    x_sb = px.tile([128, nhc, NQT], bf16, name="x_sb")
                    nc.sync.dma_start(out=x_sb[:], in_=xt_r[:, :, ts])
                    # Q^T chunks (4 x [128, NQT])
                    for m in range(4):
                        ps = pps.tile([128, NQT], f32, name="ps", tag="ps")
                        for c in range(nhc):
                            nc.tensor.matmul(
                                ps[:], wq_sb[:, c, m * 128:(m + 1) * 128],
                                x_sb[:, c, :],
                                start=(c == 0), stop=(c == nhc - 1))
                        rope_evict(ps, qb_sb[:, m:m + 1], qt[m][:, ts], ts)
                    # K^T (one [128, NQT] chunk = 2 kv heads)
                    psk = pps.tile([128, NQT], f32, name="psk", tag="ps")
                    for c in range(nhc):
                        nc.tensor.matmul(psk[:], wk_sb[:, c, :], x_sb[:, c, :],
                                         start=(c == 0), stop=(c == nhc - 1))
                    kf = ptmp.tile([128, NQT], bf16, name="kf", tag="kf")
                    rope_evict(psk, kb_sb[:, 0:1], kf[:], ts)
                    for half in (0, 1):           # kv head within chunk
                        for dsth in (0, 1):       # duplicate to both halves
                            nc.sync.dma_start(
                                out=ktd[half][dsth * 64:(dsth + 1) * 64, ts],
                                in_=kf[half * 64:(half + 1) * 64, :])
                    # V (T rows on partitions) + bias via rank-1 matmul
                    for st in range(NQT // 128):
                        psv = pps.tile([128, KVDIM], f32, name="psv", tag="psv")
                        for c in range(nhc):
                            nc.tensor.matmul(
                                psv[:], x_sb[:, c, st * 128:(st + 1) * 128],
                                wv_sb[:, c, :],
                                start=(c == 0), stop=False)
                        nc.tensor.matmul(psv[:], ones_sb[:], vb_sb[:],
                                         start=False, stop=True)
                        vi = tb * (NQT // 128) + st
                        nc.vector.memset(vsb[vi][:, :], 1.0)
                        nc.vector.tensor_copy(vsb[vi][:, 0:64], psv[:, 0:64])
                        nc.vector.tensor_copy(vsb[vi][:, 65:129], psv[:, 64:128])

            # ---------------- attention + AllGather + o_proj ----------------
            max_emt_j = max((sum(1 for e in pj if e[1] == 1) for pj in plans),
                            default=1)
            with tc.tile_pool(name="emtp", bufs=max(2, min(2 * max_emt_j, 12))) as ep, \
                 tc.tile_pool(name="ptp", bufs=4) as ptp, \
                 tc.tile_pool(name="psS", bufs=2, space="PSUM") as psS, \
                 tc.tile_pool(name="psO", bufs=1, space="PSUM") as psO, \
                 tc.tile_pool(name="evp", bufs=6) as evp, \
                 tc.tile_pool(name="psF", bufs=2, space="PSUM") as psF, \
                 tc.tile_pool(name="agp", bufs=3) as agp, \
                 tc.tile_pool(name="outp", bufs=4) as outp:
                ag_sb_tiles = {}

                def emit_attention(j):
                    qs = slice(j * NQT, (j + 1) * NQT)
                    emt_sb = {}
                    for (i, kind, gi) in plans[j]:
                        if kind == 1:
                            e = ep.tile([NKC, NQT], bf16, name="emt_sb",
                                        tag="emt")
                            nc.sync.dma_start(out=e[:], in_=emt[gi])
                            emt_sb[i] = e
                    for hp in range(4):           # head pair (2hp, 2hp+1)
                        kv = hp // 2              # local kv head
                        po0 = psO.tile([65, NQT], f32, name="po0", tag="po0")
                        po1 = psO.tile([65, NQT], f32, name="po1", tag="po1")
                        n_ch = len(plans[j])
                        for ci, (i, kind, gi) in enumerate(plans[j]):
                            pss = psS.tile([128, 1024], f32, name="pss",
                                           tag="pss")
                            # head-lo on array rows 0:64, head-hi on 64:128 —
                            # concurrent row-groups, separate PSUM banks
                            nc.tensor.matmul(
                                pss[:, 0:512],
                                ktd[kv][0:64, i * NKC:(i + 1) * NKC],
                                qt[hp][0:64, qs], start=True, stop=True)
                            nc.tensor.matmul(
                                pss[:, 512:1024],
                                ktd[kv][64:128, i * NKC:(i + 1) * NKC],
                                qt[hp][64:128, qs], start=True, stop=True)
                            pt = ptp.tile([128, 1024], bf16, name="pt",
                                          tag="pt")
                            nc.scalar.activation(pt[:], pss[:], AF.Exp,
                                                 scale=0.125)
                            if kind == 1:
                                nc.vector.tensor_mul(pt[:, 0:512],
                                                     pt[:, 0:512], emt_sb[i][:])
                                nc.vector.tensor_mul(pt[:, 512:1024],
                                                     pt[:, 512:1024],
                                                     emt_sb[i][:])
                            vsl = vsb[i][:, 0:65] if kv == 0 else vsb[i][:, 65:130]
                            nc.tensor.matmul(po0[:], vsl, pt[:, 0:512],
                                             start=(ci == 0),
                                             stop=(ci == n_ch - 1))
                            nc.tensor.matmul(po1[:], vsl, pt[:, 512:1024],
                                             start=(ci == 0),
                                             stop=(ci == n_ch - 1))
                        for s, po in enumerate((po0, po1)):
                            # One fast copy frees the PSUM bank so the next
                            # head-pair's PV can start; the divide chain then
                            # runs off the PE critical path from SBUF.
                            pocp = evp.tile([65, NQT], f32, name="pocp",
                                            tag="pocp")
                            nc.vector.tensor_copy(pocp[:], po[:])
                            rc = evp.tile([1, NQT], f32, name="rc", tag="rc")
                            nc.vector.reciprocal(rc[:], pocp[64:65, :])
                            rb = evp.tile([64, NQT], f32, name="rb", tag="rb")
                            nc.gpsimd.partition_broadcast(rb[:], rc[:])
                            at = evp.tile([64, NQT], bf16, name="at", tag="at")
                            nc.vector.tensor_mul(at[:], pocp[0:64, :], rb[:])
                            h = 2 * hp + s
                            nc.sync.dma_start(
                                out=ag_in[j][h * 64:(h + 1) * 64, :],
                                in_=at[:])
                    nc.gpsimd.collective_compute(
                        "AllGather", ALU.bypass, replica_groups=rg,
                        ins=[ag_in[j].opt()], outs=[ag_out[j].opt()])
                    ag_sb = agp.tile([128, noc, NQT], bf16, name="ag_sb")
                    nc.sync.dma_start(
                        out=ag_sb[:],
                        in_=ag_out[j].rearrange("(c p) t -> p c t", p=128))
                    ag_sb_tiles[j] = ag_sb

                def emit_oproj(j):
                    ag_sb = ag_sb_tiles.pop(j)
                    for tt in range(NQT // 128):
                        pf = psF.tile([128, QDIM], f32, name="pf", tag="pf")
                        for c in range(noc):
                            nc.tensor.matmul(
                                pf[:], ag_sb[:, c, tt * 128:(tt + 1) * 128],
                                ow_sb[:, c, :],
                                start=(c == 0), stop=(c == noc - 1))
                        ot = outp.tile([128, QDIM], f32, name="ot", tag="ot")
                        nc.vector.tensor_copy(ot[:], pf[:])
                        nc.sync.dma_start(
                            out=out[j * NQT + tt * 128:
                                    j * NQT + (tt + 1) * 128, :],
                            in_=ot[:])

                # Pipeline: o_proj(j) is emitted well after attention(j) so
                # its PE work lands while later AllGathers are in flight;
                # o_proj(1..2) deliberately fill the final AllGather's
                # latency window, leaving only o_proj(3) after it.
                if nj == 4:
                    emit_attention(0)
                    emit_attention(1)
                    emit_oproj(0)
                    emit_attention(2)
                    emit_attention(3)
                    emit_oproj(1)
                    emit_oproj(2)
                    emit_oproj(3)
                else:
                    for j in range(nj):
                        emit_attention(j)
                    for j in range(nj):
                        emit_oproj(j)

    nc.compile()
    return nc


def prep_inputs(hidden, positions, mask, q_w, q_b, k_w, k_b, v_w, v_b, o_w,
                emt_tiles):
    """Host-side shard + transform → in_maps for the 8 cores."""
    B, T, _ = hidden.shape
    pos = np.asarray(positions)[0].astype(np.float32)
    inv_freq = (1.0 / (THETA ** (np.arange(0, D, 2, dtype=np.float32) / D)))
    freqs = pos[:, None] * inv_freq[None, :]          # (T, 32)
    cos_t, sin_t = np.cos(freqs).T, np.sin(freqs).T   # (32, T)
    cos_tab = np.ascontiguousarray(np.tile(cos_t, (4, 1)), dtype=np.float32)
    ssin_tab = np.ascontiguousarray(
        np.concatenate([sin_t, -sin_t, sin_t, -sin_t], axis=0),
        dtype=np.float32)

    if emt_tiles:
        emt_arr = np.stack(emt_tiles).astype(BF16)
    else:
        emt_arr = np.zeros((1, NKC, NQT), BF16)

    xts = [np.ascontiguousarray(hidden[b].T).astype(BF16) for b in range(B)]
    in_maps = []
    for c in range(NCORES):
        b, g = c // NGROUPS, c % NGROUPS
        qsl = slice(QDIM * g, QDIM * (g + 1))
        ksl = slice(KVDIM * g, KVDIM * (g + 1))
        in_maps.append({
            "xt": xts[b],
            "wqt": np.ascontiguousarray(q_w[qsl, :].T).astype(BF16),
            "wkt": np.ascontiguousarray(k_w[ksl, :].T).astype(BF16),
            "wvt": np.ascontiguousarray(v_w[ksl, :].T).astype(BF16),
            "qb": np.asarray(q_b[qsl], np.float32).reshape(4, 128, 1),
            "kb": np.asarray(k_b[ksl], np.float32).reshape(1, 128, 1),
            "vb": np.asarray(v_b[ksl]).astype(BF16).reshape(1, KVDIM),
            "cosq": cos_tab,
            "ssin": ssin_tab,
            "emt": emt_arr,
            "owt": np.ascontiguousarray(o_w.T[:, qsl]).astype(BF16),
        })
    return in_maps


def _ensure_ntff_hook():
    """Provide antenv.axon_hooks in containers whose antenv stub lacks it,
    wiring the ctypes NTFF profiler from the injected axon boot package."""
    import sys
    import types
    try:
        from antenv.axon_hooks import get_axon_ntff_profile_hook  # noqa: F401
        return True
    except ImportError:
        pass
    try:
        import antenv
        from trn_agent_boot.trn_boot import _ntff_profile_via_ctypes
        hook = _ntff_profile_via_ctypes("/opt/axon/libaxon_pjrt.so")
        if hook is None:
            return False
        mod = types.ModuleType("antenv.axon_hooks")
        state = {"h": hook}
        mod.get_axon_ntff_profile_hook = lambda: state["h"]
        mod.set_axon_ntff_profile_hook = lambda h: state.__setitem__("h", h)
        sys.modules["antenv.axon_hooks"] = mod
        antenv.axon_hooks = mod
        return True
    except Exception:
        return False


def kernel(hidden, positions, mask, q_w, q_b, k_w, k_b, v_w, v_b, o_w):
    global LAST_RESULT
    from concourse import bass_utils

    hidden = np.asarray(hidden)
    B, T, _ = hidden.shape
    mask_key = (T, hash(np.asarray(mask).tobytes()))
    if mask_key not in _cache:
        plans, emt_tiles = plan_mask(mask, T)
        nc = build_graph(T, plans, len(emt_tiles))
        _cache[mask_key] = (nc, emt_tiles)
    nc, emt_tiles = _cache[mask_key]

    in_maps = prep_inputs(hidden, positions, mask, q_w, q_b, k_w, k_b,
                          v_w, v_b, o_w, emt_tiles)
    trace = os.environ.get("BASS_KERNEL_TRACE", "0") == "1"
    if trace:
        trace = _ensure_ntff_hook()
    res = bass_utils.run_bass_kernel_spmd(nc, in_maps,
                                          core_ids=list(range(NCORES)),
                                          trace=trace)
    LAST_RESULT = res
    out = np.zeros((B, T, HID), np.float32)
    for c in range(NCORES):
        b, g = c // NGROUPS, c % NGROUPS
        out[b, :, QDIM * g:QDIM * (g + 1)] = res.results[c]["out"]
    return out
